# revision 12
# baseline (speedup 1.0000x reference)
"""Trainium2 Bass kernel for nn_ChargeIncrementModel (3-layer EGAT GNN + MLP).

Self-contained: takes full inputs, shards across 8 NeuronCores internally
(dst-partitioned edges, chunked node blocks), runs one SPMD Bass program with
AllGather halo exchanges, and unshards the output on the host.
"""
import sys

sys.path.insert(0, "/opt/trn_rl_repo")

import numpy as np

import concourse.bass as bass
import concourse.mybir as mybir
import concourse.tile as tile
from concourse import bacc, library_config
from concourse.bass import ds, ts
from concourse.bass_utils import run_bass_kernel_spmd
from concourse.masks import make_identity

# ---------------- problem constants (hardcoded per spec) ----------------
N_NODES = 50000
N_EDGES = 800000
IN_N, IN_E = 64, 32
HN, HE, NH = 32, 32, 2
DH = HN * NH            # 64 = hidden node dim = HE * NH = edge dim
NCORES = 8

# chunk geometry
P = 128                 # nodes per chunk / partition dim
LO_TILES = 9            # 128-edge tiles reserved for "lo" edges (src core 0-3)
HI_TILES = 9
TILES = LO_TILES + HI_TILES      # 18
LO_CAP = LO_TILES * P            # 1152
HI_CAP = HI_TILES * P
ECH = TILES * P                  # 2304 edge slots per chunk
G = 2                            # chunks per gather group

F32 = mybir.dt.float32
I16 = mybir.dt.int16

SELU_L = 1.0507009873554805
SELU_A = 1.6732632423543772
E_CLAMP = 60.0

Act = mybir.ActivationFunctionType
Alu = mybir.AluOpType


# ======================================================================
# Host-side graph preprocessing
# ======================================================================
def _wrap_idx(idx):
    """[Q] int -> dma_gather layout [128, Q//16] int16."""
    q = idx.shape[0]
    a = np.ascontiguousarray(idx.astype(np.int16).reshape(q // 16, 16).T)
    return np.tile(a, (8, 1))


def _prep(src, dst):
    """Partition + chunk the graph. Returns per-core layout dicts."""
    order = np.argsort(dst, kind="stable")
    dst_s = dst[order]
    counts = np.bincount(dst, minlength=N_NODES)
    cum = np.cumsum(counts)
    # node range per core, balanced by edge count
    nb = [0]
    for c in range(1, NCORES):
        nb.append(int(np.searchsorted(cum, c * N_EDGES / NCORES)))
    nb.append(N_NODES)
    node_core = np.zeros(N_NODES, np.int32)
    for c in range(NCORES):
        node_core[nb[c]:nb[c + 1]] = c

    src_is_lo = node_core[src] < (NCORES // 2)
    # per-node lo/hi in-degree
    lo_cnt = np.bincount(dst[src_is_lo], minlength=N_NODES)
    hi_cnt = np.bincount(dst[~src_is_lo], minlength=N_NODES)
    assert lo_cnt.max() <= LO_CAP and hi_cnt.max() <= HI_CAP

    # greedy chunking per core
    core_chunks = []  # per core: list of (node_start, node_end)
    for c in range(NCORES):
        chunks = []
        s = nb[c]
        nn = llo = lhi = 0
        for n in range(nb[c], nb[c + 1]):
            if nn == P or llo + lo_cnt[n] > LO_CAP or lhi + hi_cnt[n] > HI_CAP:
                chunks.append((s, n))
                s, nn, llo, lhi = n, 0, 0, 0
            nn += 1
            llo += lo_cnt[n]
            lhi += hi_cnt[n]
        chunks.append((s, nb[c + 1]))
        core_chunks.append(chunks)

    n_chunks = max(len(ch) for ch in core_chunks)
    n_chunks = (n_chunks + G - 1) // G * G  # pad to group multiple
    s_core = n_chunks * P
    assert (NCORES // 2) * s_core <= 32768, (
        f"gather index space too large: {s_core}"
    )

    # slot maps
    slot_of_node = np.full(N_NODES, -1, np.int64)
    node_of_slot = np.full(NCORES * s_core, -1, np.int64)
    for c in range(NCORES):
        for k, (a, b) in enumerate(core_chunks[c]):
            slots = c * s_core + k * P + np.arange(b - a)
            slot_of_node[a:b] = slots
            node_of_slot[slots] = np.arange(a, b)

    # per-core edge layout
    src_slot_all = slot_of_node[src]
    cores = []
    # edge boundaries per core in dst-sorted order
    e_bounds = [0] + [int(np.searchsorted(dst_s, nb[c + 1])) for c in range(NCORES)]
    for c in range(NCORES):
        eids_c = order[e_bounds[c]:e_bounds[c + 1]]  # original edge ids, dst-sorted
        dst_c = dst[eids_c]
        n_slots = n_chunks * ECH
        eid_of_slot = np.full(n_slots, -1, np.int64)
        idx_lo = np.zeros((n_chunks, LO_CAP), np.int16)
        idx_hi = np.zeros((n_chunks, HI_CAP), np.int16)
        idx_nj = np.zeros((n_chunks, ECH), np.int16)
        dst_local = np.full((n_chunks, ECH), -1.0, np.float32)
        chunks = core_chunks[c]
        # chunk id per edge: searchsorted on chunk node starts
        ch_starts = np.array([a for a, b in chunks], np.int64)
        e_chunk = np.searchsorted(ch_starts, dst_c, side="right") - 1
        for k in range(len(chunks)):
            em = eids_c[e_chunk == k]
            if em.size == 0:
                continue
            sl = src_slot_all[em]
            lo_m = sl < (NCORES // 2) * s_core
            e_lo, e_hi = em[lo_m], em[~lo_m]
            nlo, nhi = e_lo.size, e_hi.size
            idx_lo[k, :nlo] = src_slot_all[e_lo].astype(np.int16)
            idx_hi[k, :nhi] = (src_slot_all[e_hi] - (NCORES // 2) * s_core).astype(
                np.int16
            )
            pos_lo = np.arange(nlo)
            pos_hi = LO_CAP + np.arange(nhi)
            pos = np.concatenate([pos_lo, pos_hi])
            ee = np.concatenate([e_lo, e_hi])
            eid_of_slot[k * ECH + pos] = ee
            dloc = slot_of_node[dst[ee]] - (c * s_core + k * P)
            assert (dloc >= 0).all() and (dloc < P).all()
            idx_nj[k, pos] = (slot_of_node[dst[ee]] - c * s_core).astype(np.int16)
            dst_local[k, pos] = dloc.astype(np.float32)

        # pack idx arrays into per-group wrapped layout [128, n_groups*W]
        ngr = n_chunks // G

        def pack(arr, cap):
            w = G * cap // 16
            out = np.zeros((P, ngr * w), np.int16)
            for g in range(ngr):
                q = arr[g * G:(g + 1) * G].reshape(-1)
                out[:, g * w:(g + 1) * w] = _wrap_idx(q)
            return out

        cores.append(
            dict(
                eid_of_slot=eid_of_slot,
                idx_lo=pack(idx_lo, LO_CAP),
                idx_hi=pack(idx_hi, HI_CAP),
                idx_nj=pack(idx_nj, ECH),
                # [p, k*TILES+t] = dst_local[k, t*128+p]
                dst_local=np.ascontiguousarray(
                    dst_local.reshape(n_chunks, TILES, P)
                    .transpose(2, 0, 1)
                    .reshape(P, n_chunks * TILES)
                ),
            )
        )
    return dict(
        nb=nb,
        n_chunks=n_chunks,
        s_core=s_core,
        slot_of_node=slot_of_node,
        node_of_slot=node_of_slot,
        cores=cores,
    )


# ======================================================================
# Device program
# ======================================================================
def build_program(nc, n_chunks):
    s_core = n_chunks * P
    s_total = NCORES * s_core
    e_slots = n_chunks * ECH
    ngr = n_chunks // G
    QLO, QHI, QNJ = G * LO_CAP, G * HI_CAP, G * ECH
    WLO, WHI, WNJ = QLO // 16, QHI // 16, QNJ // 16

    # ---- I/O ----
    he0 = nc.dram_tensor("he0", [IN_E, e_slots], F32, kind="ExternalInput")
    hn0 = nc.dram_tensor("hn0", [s_core, DH], F32, kind="ExternalInput")
    chg = nc.dram_tensor("chg", [P, n_chunks], F32, kind="ExternalInput")
    d_idx_lo = nc.dram_tensor("idx_lo", [P, ngr * WLO], I16, kind="ExternalInput")
    d_idx_hi = nc.dram_tensor("idx_hi", [P, ngr * WHI], I16, kind="ExternalInput")
    d_idx_nj = nc.dram_tensor("idx_nj", [P, ngr * WNJ], I16, kind="ExternalInput")
    d_dstloc = nc.dram_tensor(
        "dst_local", [P, n_chunks * TILES], F32, kind="ExternalInput"
    )
    # weights
    d_wfij = [
        nc.dram_tensor(f"wfij{l}", [IN_E if l == 0 else DH, DH], F32,
                       kind="ExternalInput")
        for l in range(3)
    ]
    d_w3 = [
        nc.dram_tensor(f"w3_{l}", [DH, 3 * DH], F32, kind="ExternalInput")
        for l in range(3)
    ]
    d_bias = [
        nc.dram_tensor(f"bias{l}", [P, DH], F32, kind="ExternalInput")
        for l in range(3)
    ]
    d_attn = [
        nc.dram_tensor(f"attn{l}", [P, TILES * DH], F32, kind="ExternalInput")
        for l in range(3)
    ]
    d_w0 = nc.dram_tensor("w0", [DH, DH], F32, kind="ExternalInput")
    d_b0 = nc.dram_tensor("b0", [DH, 1], F32, kind="ExternalInput")
    d_w1 = nc.dram_tensor("w1", [DH, 1], F32, kind="ExternalInput")
    d_b1 = nc.dram_tensor("b1", [P, 1], F32, kind="ExternalInput")

    out_chg = nc.dram_tensor("out_chg", [P, n_chunks], F32, kind="ExternalOutput")
    out_inc = nc.dram_tensor(
        "out_inc", [P, n_chunks * TILES], F32, kind="ExternalOutput"
    )

    # ---- internal DRAM ----
    he_nxt = [
        None,
        nc.dram_tensor("he1", [DH, e_slots], F32),
        nc.dram_tensor("he2", [DH, e_slots], F32),
    ]
    stage = [nc.dram_tensor(f"stage{l}", [s_core, 2 * DH], F32) for l in range(3)]
    import os
    _shared = {} if os.environ.get("KERNEL_NOSHARED") else {"addr_space": "Shared"}
    full = [
        nc.dram_tensor(f"full{l}", [s_total, 2 * DH], F32, **_shared)
        for l in range(3)
    ]
    fnj = [nc.dram_tensor(f"fnj{l}", [s_core, DH], F32) for l in range(3)]

    groups = [list(range(NCORES))]

    with tile.TileContext(nc) as tc:
        with (
            tc.tile_pool(name="cst", bufs=1) as cst,
            tc.tile_pool(name="idx", bufs=2) as idxp,
            tc.tile_pool(name="gbuf", bufs=2) as gb,
            tc.tile_pool(name="work", bufs=2) as wk,
            tc.tile_pool(name="small", bufs=2) as sm,
            tc.tile_pool(name="ohp", bufs=3) as ohp,
            tc.tile_pool(name="pfs", bufs=3, space="PSUM") as pfs,
            tc.tile_pool(name="pscat", bufs=2, space="PSUM") as pscat,
            tc.tile_pool(name="pdel", bufs=1, space="PSUM") as pdel,
            tc.tile_pool(name="paux", bufs=2, space="PSUM") as paux,
        ):
            nc.gpsimd.load_library(library_config.mlp)

            # ---- constants ----
            ident = cst.tile([P, P], F32)
            make_identity(nc, ident[:])
            iota = cst.tile([P, P], F32)
            nc.gpsimd.iota(
                iota[:], pattern=[[1, P]], base=0, channel_multiplier=0,
                allow_small_or_imprecise_dtypes=True,
            )
            dstloc = cst.tile([P, n_chunks * TILES], F32)
            nc.sync.dma_start(dstloc[:], d_dstloc[:])
            chg_sb = cst.tile([P, n_chunks], F32)
            nc.sync.dma_start(chg_sb[:], chg[:])
            wfij = []
            for l in range(3):
                t = cst.tile([IN_E if l == 0 else DH, DH], F32, tag=f"wfij{l}")
                nc.sync.dma_start(t[:], d_wfij[l][:])
                wfij.append(t)
            w3 = []
            for l in range(3):
                t = cst.tile([DH, 3 * DH], F32, tag=f"w3_{l}")
                nc.sync.dma_start(t[:], d_w3[l][:])
                w3.append(t)
            bias = []
            for l in range(3):
                t = cst.tile([P, DH], F32, tag=f"bias{l}")
                nc.sync.dma_start(t[:], d_bias[l][:])
                bias.append(t)
            attn_t = cst.tile([P, TILES * DH], F32, tag="attn")
            w0 = cst.tile([DH, DH], F32)
            nc.sync.dma_start(w0[:], d_w0[:])
            b0 = cst.tile([DH, 1], F32)
            nc.sync.dma_start(b0[:], d_b0[:])
            w1 = cst.tile([DH, 1], F32)
            nc.sync.dma_start(w1[:], d_w1[:])
            b1 = cst.tile([P, 1], F32)
            nc.sync.dma_start(b1[:], d_b1[:])

            out_all = cst.tile([P, n_chunks], F32)
            nc.vector.memset(out_all[:], 0.0)

            # ---- helper: project hn chunk -> stage rows ----
            def project_and_stage(hn_t, k_expr, lyr):
                """hn_t: [P, DH] sbuf (node-major chunk of hn). Writes stage/fnj
                rows [k*P, (k+1)*P) for layer `lyr` (table feeding layer lyr)."""
                tp = pfs.tile([DH, P], F32, tag="pe")
                nc.tensor.transpose(tp[:], hn_t[:], ident[:])
                hnT = sm.tile([DH, P], F32, tag="hnT")
                nc.scalar.activation(hnT[:], tp[:], Act.Copy)
                pp = paux.tile([P, 3 * DH], F32, tag="paux")
                nc.tensor.matmul(pp[:], lhsT=hnT[:], rhs=w3[lyr][:],
                                 start=True, stop=True)
                st = sm.tile([P, 3 * DH], F32, tag="stage_sb")
                nc.scalar.activation(st[:, 0:2 * DH], pp[:, 0:2 * DH], Act.Copy)
                nc.vector.tensor_add(
                    out=st[:, 2 * DH:3 * DH], in0=pp[:, 2 * DH:3 * DH],
                    in1=bias[lyr][:],
                )
                nc.sync.dma_start(stage[lyr][ds(k_expr * P, P), :], st[:, 0:2 * DH])
                nc.sync.dma_start(fnj[lyr][ds(k_expr * P, P), :],
                                  st[:, 2 * DH:3 * DH])

            # ---- layer-0 node phase ----
            for k in range(n_chunks):
                hn_t = sm.tile([P, DH], F32, tag="hn0_t")
                nc.sync.dma_start(hn_t[:], hn0[ts(k, P), :])
                project_and_stage(hn_t, k, 0)

            nc.gpsimd.collective_compute(
                "AllGather", Alu.bypass, replica_groups=groups,
                ins=[stage[0][:]], outs=[full[0][:]],
            )

            # ---- 3 EGAT layers ----
            _nl = int(os.environ.get("KERNEL_NLAYERS", "3"))
            for l in range(_nl):
                de = IN_E if l == 0 else DH
                he_src = he0 if l == 0 else he_nxt[l]
                nc.sync.dma_start(attn_t[:], d_attn[l][:])

                def body(g, l=l, de=de, he_src=he_src):
                    # --- group loads ---
                    ilo = idxp.tile([P, WLO], I16, tag="ilo")
                    nc.sync.dma_start(ilo[:], d_idx_lo[:, ts(g, WLO)])
                    ihi = idxp.tile([P, WHI], I16, tag="ihi")
                    nc.sync.dma_start(ihi[:], d_idx_hi[:, ts(g, WHI)])
                    inj = idxp.tile([P, WNJ], I16, tag="inj")
                    nc.sync.dma_start(inj[:], d_idx_nj[:, ts(g, WNJ)])

                    glo = gb.tile([P, G * LO_TILES, 2 * DH], F32, tag="glo")
                    nc.gpsimd.dma_gather(
                        glo[:], full[l][0:(NCORES // 2) * s_core, :], ilo[:],
                        QLO, QLO, 2 * DH, single_packet=False,
                    )
                    ghi = gb.tile([P, G * HI_TILES, 2 * DH], F32, tag="ghi")
                    nc.gpsimd.dma_gather(
                        ghi[:], full[l][(NCORES // 2) * s_core:, :], ihi[:],
                        QHI, QHI, 2 * DH, single_packet=False,
                    )
                    gnj = gb.tile([P, G * TILES, DH], F32, tag="gnj")
                    nc.gpsimd.dma_gather(
                        gnj[:], fnj[l][:], inj[:], QNJ, QNJ, DH,
                        single_packet=False,
                    )
                    for cc in range(G):
                        heT = wk.tile([de, ECH], F32, tag="heT")
                        nc.sync.dma_start(
                            heT[:], he_src[:, ds((g * G + cc) * ECH, ECH)]
                        )
                        f_sb = wk.tile([P, TILES * DH], F32, tag="f_sb")
                        fT = wk.tile([DH, ECH], F32, tag="fT")
                        # ---------- loop A: per-tile f ----------
                        for t in range(TILES):
                            fps = pfs.tile([P, DH], F32, tag="pe")
                            nc.tensor.matmul(
                                fps[:],
                                lhsT=heT[:, t * P:(t + 1) * P],
                                rhs=wfij[l][:], start=True, stop=False,
                            )
                            if t < LO_TILES:
                                gt = glo[:, cc * LO_TILES + t, :]
                            else:
                                gt = ghi[:, cc * HI_TILES + (t - LO_TILES), :]
                            nc.tensor.matmul(fps[:], lhsT=ident[:],
                                             rhs=gt[:, 0:DH],
                                             start=False, stop=False)
                            nc.tensor.matmul(
                                fps[:], lhsT=ident[:],
                                rhs=gnj[:, cc * TILES + t, :],
                                start=False, stop=True,
                            )
                            fsl = f_sb[:, t * DH:(t + 1) * DH]
                            # fsl = -leaky_relu(x) = 0.99*relu(-x) - x
                            rneg = sm.tile([P, DH], F32, tag="rneg")
                            nc.scalar.activation(rneg[:], fps[:], Act.Relu,
                                                 scale=-1.0)
                            nc.vector.scalar_tensor_tensor(
                                out=fsl, in0=rneg[:], scalar=-0.99, in1=fps[:],
                                op0=Alu.mult, op1=Alu.subtract,
                            )
                            # he_next = relu(x) = rneg + fsl ... no: relu(x) =
                            # x + relu(-x) - ... use relu(x) = fsl_neg? direct:
                            relp = sm.tile([P, DH], F32, tag="relp")
                            nc.scalar.activation(relp[:], fps[:], Act.Relu)
                            tp = pfs.tile([DH, P], F32, tag="pe")
                            nc.tensor.transpose(tp[:], relp[:], ident[:])
                            nc.scalar.activation(
                                fT[:, t * P:(t + 1) * P], tp[:], Act.Copy,
                            )

                        # ---------- chunk-wide attention ----------
                        emul = sm.tile([P, TILES * DH], F32, tag="emul")
                        nc.vector.tensor_mul(
                            out=emul[:],
                            in0=f_sb[:],
                            in1=attn_t[:],
                        )
                        e_sb = sm.tile([P, TILES * NH], F32, tag="e_sb")
                        nc.vector.reduce_sum(
                            e_sb[:],
                            emul[:].rearrange("p (a b) -> p a b", b=HE),
                            axis=mybir.AxisListType.X,
                        )
                        nc.vector.tensor_scalar(
                            out=e_sb[:], in0=e_sb[:], scalar1=E_CLAMP,
                            scalar2=None, op0=Alu.min,
                        )
                        payload = sm.tile([P, TILES * (2 + DH)], F32, tag="payload")
                        pay3 = payload[:].rearrange(
                            "p (t f) -> p t f", f=2 + DH)
                        nc.scalar.activation(
                            pay3[:, :, 0:2],
                            e_sb[:].rearrange("p (t h) -> p t h", h=NH), Act.Exp
                        )
                        # wmsg per head (broadcast ex over 32 cols)
                        for t in range(TILES):
                            gt = (glo[:, cc * LO_TILES + t, :] if t < LO_TILES
                                  else ghi[:, cc * HI_TILES + (t - LO_TILES), :])
                            for h in range(NH):
                                nc.vector.tensor_mul(
                                    out=pay3[:, t, 2 + h * HE:2 + (h + 1) * HE],
                                    in0=gt[:, DH + h * HE:DH + (h + 1) * HE],
                                    in1=pay3[:, t, h:h + 1].to_broadcast([P, HE]),
                                )

                        # ---------- layer-2 MLP on fT ----------
                        if l == 2:
                            hidT = sm.tile([DH, ECH], F32, tag="hidT")
                            for off in range(0, ECH, 512):
                                w = min(512, ECH - off)
                                hp = paux.tile([DH, 512], F32, tag="paux")
                                nc.tensor.matmul(
                                    hp[:, :w], lhsT=w0[:],
                                    rhs=fT[:, off:off + w],
                                    start=True, stop=True,
                                )
                                t1 = sm.tile([DH, 512], F32, tag="t1")
                                nc.scalar.activation(
                                    t1[:, :w], hp[:, :w], Act.Exp, bias=b0[:]
                                )
                                t2 = sm.tile([DH, 512], F32, tag="t2")
                                nc.scalar.activation(
                                    t2[:, :w], hp[:, :w], Act.Relu, bias=b0[:]
                                )
                                r = sm.tile([DH, 512], F32, tag="r")
                                nc.scalar.activation(
                                    r[:, :w], t1[:, :w], Act.Relu,
                                    scale=-1.0, bias=1.0,
                                )
                                dtmp = sm.tile([DH, 512], F32, tag="dtmp")
                                nc.vector.scalar_tensor_tensor(
                                    out=dtmp[:, :w], in0=r[:, :w], scalar=SELU_A,
                                    in1=t2[:, :w], op0=Alu.mult, op1=Alu.subtract,
                                )
                                nc.vector.tensor_scalar(
                                    out=hidT[:, off:off + w], in0=dtmp[:, :w],
                                    scalar1=-SELU_L, scalar2=None, op0=Alu.mult,
                                )
                            inc_sb = sm.tile([P, TILES], F32, tag="inc_sb")

                        # ---------- loop B: scatter ----------
                        psc = pscat.tile([P, 2 + DH], F32, tag="psc")
                        if l == 2:
                            pD = pdel.tile([P, 1], F32, tag="pD")
                        for t in range(TILES):
                            oh = ohp.tile([P, P], F32, tag="oh")
                            col = g * (G * TILES) + cc * TILES + t
                            nc.vector.tensor_tensor(
                                out=oh[:],
                                in0=dstloc[:, ds(col, 1)].to_broadcast([P, P]),
                                in1=iota[:], op=Alu.is_equal,
                            )
                            nc.tensor.matmul(
                                psc[:], lhsT=oh[:],
                                rhs=payload[:, t * (2 + DH):(t + 1) * (2 + DH)],
                                start=(t == 0), stop=(t == TILES - 1),
                            )
                            if l == 2:
                                ip = paux.tile([P, 1], F32, tag="paux")
                                nc.tensor.matmul(
                                    ip[:], lhsT=hidT[:, t * P:(t + 1) * P],
                                    rhs=w1[:], start=True, stop=True,
                                )
                                nc.vector.tensor_scalar(
                                    out=inc_sb[:, t:t + 1], in0=ip[:],
                                    scalar1=b1[:, 0:1], scalar2=None, op0=Alu.add,
                                )
                                nc.tensor.matmul(
                                    pD[:], lhsT=oh[:], rhs=inc_sb[:, t:t + 1],
                                    start=(t == 0), stop=(t == TILES - 1),
                                )

                        # ---------- chunk tail ----------
                        if l < 2:
                            zt = sm.tile([P, 2], F32, tag="zt")
                            nc.vector.tensor_scalar(
                                out=zt[:], in0=psc[:, 0:2], scalar1=0.0,
                                scalar2=None, op0=Alu.is_equal,
                            )
                            s_sb = sm.tile([P, 2], F32, tag="s_sb")
                            nc.vector.tensor_add(out=s_sb[:], in0=psc[:, 0:2],
                                                 in1=zt[:])
                            rec = sm.tile([P, 2], F32, tag="rec")
                            nc.vector.reciprocal(rec[:], s_sb[:])
                            hr = sm.tile([P, DH], F32, tag="hr")
                            nc.scalar.activation(hr[:], psc[:, 2:2 + DH], Act.Relu)
                            hn_t = sm.tile([P, DH], F32, tag="hn_t")
                            for h in range(NH):
                                nc.vector.tensor_mul(
                                    out=hn_t[:, h * HN:(h + 1) * HN],
                                    in0=hr[:, h * HN:(h + 1) * HN],
                                    in1=rec[:, h:h + 1].to_broadcast([P, HN]),
                                )
                            project_and_stage(hn_t, g * G + cc, l + 1)
                        else:
                            oc = sm.tile([P, 1], F32, tag="oc")
                            nc.vector.tensor_add(
                                out=oc[:], in0=pD[:],
                                in1=chg_sb[:, ds(g * G + cc, 1)],
                            )
                            nc.vector.tensor_copy(
                                out=out_all[:, ds(g * G + cc, 1)], in_=oc[:]
                            )
                            nc.sync.dma_start(
                                out_inc[:, ds((g * G + cc) * TILES, TILES)],
                                inc_sb[:],
                            )

                        if l < 2:
                            nc.sync.dma_start(
                                he_nxt[l + 1][:, ds((g * G + cc) * ECH, ECH)],
                                fT[:],
                            )

                with tc.For_i(0, ngr, 1) as g:
                    body(g)

                if l < 2:
                    nc.gpsimd.collective_compute(
                        "AllGather", Alu.bypass, replica_groups=groups,
                        ins=[stage[l + 1][:]], outs=[full[l + 1][:]],
                    )

            nc.sync.dma_start(out_chg[:], out_all[:])

    return nc


# ======================================================================
# Entry point
# ======================================================================
_CACHE = {}


def kernel(feats_node, feats_edge, charges_init, src, dst, params):
    feats_node = np.asarray(feats_node, np.float32)
    feats_edge = np.asarray(feats_edge, np.float32)
    charges_init = np.asarray(charges_init, np.float32)
    src_i = np.asarray(src).astype(np.int64)
    dst_i = np.asarray(dst).astype(np.int64)
    params = [
        {k: np.asarray(v, np.float32) for k, v in p.items()} for p in params
    ]

    lay = _prep(src_i, dst_i)
    n_chunks, s_core = lay["n_chunks"], lay["s_core"]
    e_slots = n_chunks * ECH

    # ---- shared weight arrays ----
    wmaps = {}
    for l in range(3):
        p = params[l]
        wmaps[f"wfij{l}"] = np.ascontiguousarray(p["fc_fij"])
        wmaps[f"w3_{l}"] = np.ascontiguousarray(
            np.concatenate([p["fc_ni"], p["fc_node"], p["fc_nj"]], axis=1)
        )
        wmaps[f"bias{l}"] = np.tile(p["bias"][None, :], (P, 1)).astype(np.float32)
        attn_flat = -p["attn"].reshape(NH * HE)  # negated: f_sb holds -lrelu
        wmaps[f"attn{l}"] = np.tile(attn_flat[None, :], (P, TILES)).astype(
            np.float32
        )
    mlp = params[3]
    wmaps["w0"] = mlp["w0"]
    wmaps["b0"] = mlp["b0"].reshape(DH, 1)
    wmaps["w1"] = mlp["w1"].reshape(DH, 1)
    wmaps["b1"] = np.tile(mlp["b1"].reshape(1, 1), (P, 1))

    # ---- per-core arrays ----
    in_maps = []
    for c in range(NCORES):
        co = lay["cores"][c]
        eid = co["eid_of_slot"]
        val = eid >= 0
        he0 = np.zeros((e_slots, IN_E), np.float32)
        he0[val] = feats_edge[eid[val]]
        hn0 = np.zeros((s_core, DH), np.float32)
        nos = lay["node_of_slot"][c * s_core:(c + 1) * s_core]
        nval = nos >= 0
        hn0[nval] = feats_node[nos[nval]]
        chg_arr = np.zeros((s_core,), np.float32)
        chg_arr[nval] = charges_init[nos[nval]]
        m = dict(wmaps)
        m["he0"] = np.ascontiguousarray(he0.T)
        m["hn0"] = hn0
        # chg layout [p, k] = slot k*P+p
        m["chg"] = np.ascontiguousarray(chg_arr.reshape(n_chunks, P).T)
        m["idx_lo"] = co["idx_lo"]
        m["idx_hi"] = co["idx_hi"]
        m["idx_nj"] = co["idx_nj"]
        m["dst_local"] = co["dst_local"]
        in_maps.append(m)

    # ---- build + compile (cache on n_chunks) ----
    if n_chunks not in _CACHE:
        nc = bacc.Bacc("TRN2", target_bir_lowering=False, debug=False,
                       num_devices=NCORES)
        build_program(nc, n_chunks)
        nc.compile()
        _CACHE[n_chunks] = nc
    nc = _CACHE[n_chunks]

    import os as _os
    _trace = bool(_os.environ.get("KERNEL_TRACE"))
    res = run_bass_kernel_spmd(
        nc, in_maps, core_ids=list(range(NCORES)), trace=_trace,
        tmpdir=_os.environ.get("KERNEL_TRACE_DIR") or None,
    )
    if _trace and res.exec_time_ns is not None:
        print(f"HW exec time: {res.exec_time_ns} ns")

    # ---- unshard ----
    out = np.zeros(N_NODES, np.float32)
    inc_full = np.zeros(N_EDGES, np.float32)
    for c in range(NCORES):
        r = res.results[c]
        co = lay["cores"][c]
        # out_chg [P, n_chunks] -> slots k*P+p
        oc = np.ascontiguousarray(r["out_chg"].T).reshape(-1)  # [s_core]
        nos = lay["node_of_slot"][c * s_core:(c + 1) * s_core]
        nval = nos >= 0
        out[nos[nval]] = oc[nval]
        # out_inc [P, n_chunks*TILES]: [p, k*TILES+t] = edge slot k*ECH+t*P+p
        oi = r["out_inc"].reshape(P, n_chunks, TILES).transpose(1, 2, 0).reshape(-1)
        eid = co["eid_of_slot"]
        ev = eid >= 0
        inc_full[eid[ev]] = oi[ev]

    out -= np.bincount(src_i, weights=inc_full, minlength=N_NODES).astype(
        np.float32
    )
    return out


# revision 16
# speedup vs baseline: 1.8530x; 1.8530x over previous
"""Trainium2 Bass kernel for nn_ChargeIncrementModel (3-layer EGAT GNN + MLP).

Self-contained: takes full inputs, shards across 8 NeuronCores internally
(dst-partitioned edges, chunked node blocks), runs one SPMD Bass program with
AllGather halo exchanges, and unshards the output on the host.
"""
import sys

sys.path.insert(0, "/opt/trn_rl_repo")

import numpy as np
import ml_dtypes

import concourse.bass as bass
import concourse.mybir as mybir
import concourse.tile as tile
from concourse import bacc, library_config
from concourse.bass import ds, ts
from concourse.bass_utils import run_bass_kernel_spmd
from concourse.masks import make_identity

# ---------------- problem constants (hardcoded per spec) ----------------
N_NODES = 50000
N_EDGES = 800000
IN_N, IN_E = 64, 32
HN, HE, NH = 32, 32, 2
DH = HN * NH            # 64 = hidden node dim = HE * NH = edge dim
NCORES = 8

# chunk geometry
P = 128                 # nodes per chunk / partition dim
LO_TILES = 9            # 128-edge tiles reserved for "lo" edges (src core 0-3)
HI_TILES = 9
TILES = LO_TILES + HI_TILES      # 18
LO_CAP = LO_TILES * P            # 1152
HI_CAP = HI_TILES * P
ECH = TILES * P                  # 2304 edge slots per chunk
G = 2                            # chunks per gather group

F32 = mybir.dt.float32
BF = mybir.dt.bfloat16
I16 = mybir.dt.int16

SELU_L = 1.0507009873554805
SELU_A = 1.6732632423543772
E_CLAMP = 60.0

Act = mybir.ActivationFunctionType
Alu = mybir.AluOpType


# ======================================================================
# Host-side graph preprocessing
# ======================================================================
def _wrap_idx(idx):
    """[Q] int -> dma_gather layout [128, Q//16] int16."""
    q = idx.shape[0]
    a = np.ascontiguousarray(idx.astype(np.int16).reshape(q // 16, 16).T)
    return np.tile(a, (8, 1))


def _prep(src, dst):
    """Partition + chunk the graph. Returns per-core layout dicts."""
    order = np.argsort(dst, kind="stable")
    dst_s = dst[order]
    counts = np.bincount(dst, minlength=N_NODES)
    cum = np.cumsum(counts)
    # node range per core, balanced by edge count
    nb = [0]
    for c in range(1, NCORES):
        nb.append(int(np.searchsorted(cum, c * N_EDGES / NCORES)))
    nb.append(N_NODES)
    node_core = np.zeros(N_NODES, np.int32)
    for c in range(NCORES):
        node_core[nb[c]:nb[c + 1]] = c

    src_is_lo = node_core[src] < (NCORES // 2)
    # per-node lo/hi in-degree
    lo_cnt = np.bincount(dst[src_is_lo], minlength=N_NODES)
    hi_cnt = np.bincount(dst[~src_is_lo], minlength=N_NODES)
    assert lo_cnt.max() <= LO_CAP and hi_cnt.max() <= HI_CAP

    # greedy chunking per core
    core_chunks = []  # per core: list of (node_start, node_end)
    for c in range(NCORES):
        chunks = []
        s = nb[c]
        nn = llo = lhi = 0
        for n in range(nb[c], nb[c + 1]):
            if nn == P or llo + lo_cnt[n] > LO_CAP or lhi + hi_cnt[n] > HI_CAP:
                chunks.append((s, n))
                s, nn, llo, lhi = n, 0, 0, 0
            nn += 1
            llo += lo_cnt[n]
            lhi += hi_cnt[n]
        chunks.append((s, nb[c + 1]))
        core_chunks.append(chunks)

    n_chunks = max(len(ch) for ch in core_chunks)
    n_chunks = (n_chunks + G - 1) // G * G  # pad to group multiple
    s_core = n_chunks * P
    assert (NCORES // 2) * s_core <= 32768, (
        f"gather index space too large: {s_core}"
    )

    # slot maps
    slot_of_node = np.full(N_NODES, -1, np.int64)
    node_of_slot = np.full(NCORES * s_core, -1, np.int64)
    for c in range(NCORES):
        for k, (a, b) in enumerate(core_chunks[c]):
            slots = c * s_core + k * P + np.arange(b - a)
            slot_of_node[a:b] = slots
            node_of_slot[slots] = np.arange(a, b)

    # per-core edge layout
    src_slot_all = slot_of_node[src]
    cores = []
    # edge boundaries per core in dst-sorted order
    e_bounds = [0] + [int(np.searchsorted(dst_s, nb[c + 1])) for c in range(NCORES)]
    for c in range(NCORES):
        eids_c = order[e_bounds[c]:e_bounds[c + 1]]  # original edge ids, dst-sorted
        dst_c = dst[eids_c]
        n_slots = n_chunks * ECH
        eid_of_slot = np.full(n_slots, -1, np.int64)
        idx_lo = np.zeros((n_chunks, LO_CAP), np.int16)
        idx_hi = np.zeros((n_chunks, HI_CAP), np.int16)
        idx_nj = np.zeros((n_chunks, ECH), np.int16)
        dst_local = np.full((n_chunks, ECH), -1.0, np.float32)
        chunks = core_chunks[c]
        # chunk id per edge: searchsorted on chunk node starts
        ch_starts = np.array([a for a, b in chunks], np.int64)
        e_chunk = np.searchsorted(ch_starts, dst_c, side="right") - 1
        for k in range(len(chunks)):
            em = eids_c[e_chunk == k]
            if em.size == 0:
                continue
            sl = src_slot_all[em]
            lo_m = sl < (NCORES // 2) * s_core
            e_lo, e_hi = em[lo_m], em[~lo_m]
            nlo, nhi = e_lo.size, e_hi.size
            idx_lo[k, :nlo] = src_slot_all[e_lo].astype(np.int16)
            idx_hi[k, :nhi] = (src_slot_all[e_hi] - (NCORES // 2) * s_core).astype(
                np.int16
            )
            pos_lo = np.arange(nlo)
            pos_hi = LO_CAP + np.arange(nhi)
            pos = np.concatenate([pos_lo, pos_hi])
            ee = np.concatenate([e_lo, e_hi])
            eid_of_slot[k * ECH + pos] = ee
            dloc = slot_of_node[dst[ee]] - (c * s_core + k * P)
            assert (dloc >= 0).all() and (dloc < P).all()
            idx_nj[k, pos] = (slot_of_node[dst[ee]] - c * s_core).astype(np.int16)
            dst_local[k, pos] = dloc.astype(np.float32)

        # pack idx arrays into per-group wrapped layout [128, n_groups*W]
        ngr = n_chunks // G

        def pack(arr, cap):
            w = G * cap // 16
            out = np.zeros((P, ngr * w), np.int16)
            for g in range(ngr):
                q = arr[g * G:(g + 1) * G].reshape(-1)
                out[:, g * w:(g + 1) * w] = _wrap_idx(q)
            return out

        cores.append(
            dict(
                eid_of_slot=eid_of_slot,
                idx_lo=pack(idx_lo, LO_CAP),
                idx_hi=pack(idx_hi, HI_CAP),
                idx_nj=pack(idx_nj, ECH),
                # [p, k*TILES+t] = dst_local[k, t*128+p]
                dst_local=np.ascontiguousarray(
                    dst_local.reshape(n_chunks, TILES, P)
                    .transpose(2, 0, 1)
                    .reshape(P, n_chunks * TILES)
                ),
            )
        )
    return dict(
        nb=nb,
        n_chunks=n_chunks,
        s_core=s_core,
        slot_of_node=slot_of_node,
        node_of_slot=node_of_slot,
        cores=cores,
    )


# ======================================================================
# Device program
# ======================================================================
def build_program(nc, n_chunks):
    s_core = n_chunks * P
    s_total = NCORES * s_core
    e_slots = n_chunks * ECH
    ngr = n_chunks // G
    QLO, QHI, QNJ = G * LO_CAP, G * HI_CAP, G * ECH
    WLO, WHI, WNJ = QLO // 16, QHI // 16, QNJ // 16

    # ---- I/O ----
    he0 = nc.dram_tensor("he0", [IN_E, e_slots], BF, kind="ExternalInput")
    hn0 = nc.dram_tensor("hn0", [s_core, DH], BF, kind="ExternalInput")
    chg = nc.dram_tensor("chg", [P, n_chunks], F32, kind="ExternalInput")
    d_idx_lo = nc.dram_tensor("idx_lo", [P, ngr * WLO], I16, kind="ExternalInput")
    d_idx_hi = nc.dram_tensor("idx_hi", [P, ngr * WHI], I16, kind="ExternalInput")
    d_idx_nj = nc.dram_tensor("idx_nj", [P, ngr * WNJ], I16, kind="ExternalInput")
    d_dstloc = nc.dram_tensor(
        "dst_local", [P, n_chunks * TILES], F32, kind="ExternalInput"
    )
    # weights
    d_wfij = [
        nc.dram_tensor(f"wfij{l}", [IN_E if l == 0 else DH, DH], BF,
                       kind="ExternalInput")
        for l in range(3)
    ]
    d_w3 = [
        nc.dram_tensor(f"w3_{l}", [DH, 3 * DH], BF, kind="ExternalInput")
        for l in range(3)
    ]
    d_bias = [
        nc.dram_tensor(f"bias{l}", [P, DH], F32, kind="ExternalInput")
        for l in range(3)
    ]
    d_attn = [
        nc.dram_tensor(f"attn{l}", [P, TILES * DH], F32, kind="ExternalInput")
        for l in range(3)
    ]
    d_w0 = nc.dram_tensor("w0", [DH, DH], BF, kind="ExternalInput")
    d_b0 = nc.dram_tensor("b0", [DH, 1], F32, kind="ExternalInput")
    d_w1 = nc.dram_tensor("w1", [DH, 1], BF, kind="ExternalInput")
    d_b1 = nc.dram_tensor("b1", [P, 1], F32, kind="ExternalInput")

    out_chg = nc.dram_tensor("out_chg", [P, n_chunks], F32, kind="ExternalOutput")
    out_inc = nc.dram_tensor(
        "out_inc", [P, n_chunks * TILES], F32, kind="ExternalOutput"
    )

    # ---- internal DRAM ----
    he_nxt = [
        None,
        nc.dram_tensor("he1", [DH, e_slots], BF),
        nc.dram_tensor("he2", [DH, e_slots], BF),
    ]
    stage = [nc.dram_tensor(f"stage{l}", [s_core, 2 * DH], BF) for l in range(3)]
    import os
    _shared = {} if os.environ.get("KERNEL_NOSHARED") else {"addr_space": "Shared"}
    full = [
        nc.dram_tensor(f"full{l}", [s_total, 2 * DH], BF, **_shared)
        for l in range(3)
    ]
    fnj = [nc.dram_tensor(f"fnj{l}", [s_core, 2 * DH], BF) for l in range(3)]

    groups = [list(range(NCORES))]

    with tile.TileContext(nc) as tc:
        with (
            tc.tile_pool(name="cst", bufs=1) as cst,
            tc.tile_pool(name="idx", bufs=2) as idxp,
            tc.tile_pool(name="gbuf", bufs=2) as gb,
            tc.tile_pool(name="work", bufs=2) as wk,
            tc.tile_pool(name="small", bufs=2) as sm,
            tc.tile_pool(name="ohp", bufs=3) as ohp,
            tc.tile_pool(name="pfs", bufs=3, space="PSUM") as pfs,
            tc.tile_pool(name="pscat", bufs=2, space="PSUM") as pscat,
            tc.tile_pool(name="pdel", bufs=1, space="PSUM") as pdel,
            tc.tile_pool(name="paux", bufs=2, space="PSUM") as paux,
        ):
            nc.gpsimd.load_library(library_config.mlp)

            # ---- constants ----
            ident = cst.tile([P, P], BF)
            make_identity(nc, ident[:])
            iota = cst.tile([P, P], F32)
            nc.gpsimd.iota(
                iota[:], pattern=[[1, P]], base=0, channel_multiplier=0,
                allow_small_or_imprecise_dtypes=True,
            )
            dstloc = cst.tile([P, n_chunks * TILES], F32)
            nc.sync.dma_start(dstloc[:], d_dstloc[:])
            chg_sb = cst.tile([P, n_chunks], F32)
            nc.sync.dma_start(chg_sb[:], chg[:])
            wfij = []
            for l in range(3):
                t = cst.tile([IN_E if l == 0 else DH, DH], BF, tag=f"wfij{l}")
                nc.sync.dma_start(t[:], d_wfij[l][:])
                wfij.append(t)
            w3 = []
            for l in range(3):
                t = cst.tile([DH, 3 * DH], BF, tag=f"w3_{l}")
                nc.sync.dma_start(t[:], d_w3[l][:])
                w3.append(t)
            bias = []
            for l in range(3):
                t = cst.tile([P, DH], F32, tag=f"bias{l}")
                nc.sync.dma_start(t[:], d_bias[l][:])
                bias.append(t)
            attn_t = cst.tile([P, TILES * DH], F32, tag="attn")
            w0 = cst.tile([DH, DH], BF)
            nc.sync.dma_start(w0[:], d_w0[:])
            b0 = cst.tile([DH, 1], F32)
            nc.sync.dma_start(b0[:], d_b0[:])
            w1 = cst.tile([DH, 1], BF)
            nc.sync.dma_start(w1[:], d_w1[:])
            b1 = cst.tile([P, 1], F32)
            nc.sync.dma_start(b1[:], d_b1[:])

            out_all = cst.tile([P, n_chunks], F32)
            nc.vector.memset(out_all[:], 0.0)

            # zero-fill fnj pad halves (gather reads 2*DH-wide rows)
            zpad = cst.tile([P, 2 * DH], BF)
            nc.vector.memset(zpad[:], 0.0)
            for l3 in range(3):
                for k3 in range(n_chunks):
                    nc.sync.dma_start(fnj[l3][ts(k3, P), :], zpad[:])

            # ---- helper: project hn chunk -> stage rows ----
            def project_and_stage(hn_t, k_expr, lyr):
                """hn_t: [P, DH] sbuf (node-major chunk of hn). Writes stage/fnj
                rows [k*P, (k+1)*P) for layer `lyr` (table feeding layer lyr)."""
                tp = pfs.tile([DH, P], BF, tag="pe")
                nc.tensor.transpose(tp[:], hn_t[:], ident[:])
                hnT = sm.tile([DH, P], BF, tag="hnT")
                nc.scalar.activation(hnT[:], tp[:], Act.Copy)
                pp = paux.tile([P, 3 * DH], F32, tag="paux")
                nc.tensor.matmul(pp[:], lhsT=hnT[:], rhs=w3[lyr][:],
                                 start=True, stop=True)
                st = sm.tile([P, 3 * DH], BF, tag="stage_sb")
                nc.scalar.activation(st[:, 0:2 * DH], pp[:, 0:2 * DH], Act.Copy)
                nc.vector.tensor_add(
                    out=st[:, 2 * DH:3 * DH], in0=pp[:, 2 * DH:3 * DH],
                    in1=bias[lyr][:],
                )
                nc.sync.dma_start(stage[lyr][ds(k_expr * P, P), :], st[:, 0:2 * DH])
                nc.sync.dma_start(fnj[lyr][ds(k_expr * P, P), 0:DH],
                                  st[:, 2 * DH:3 * DH])

            # ---- layer-0 node phase ----
            for k in range(n_chunks):
                hn_t = sm.tile([P, DH], BF, tag="hn0_t")
                nc.sync.dma_start(hn_t[:], hn0[ts(k, P), :])
                project_and_stage(hn_t, k, 0)

            nc.gpsimd.collective_compute(
                "AllGather", Alu.bypass, replica_groups=groups,
                ins=[stage[0][:]], outs=[full[0][:]],
            )

            # ---- 3 EGAT layers ----
            _nl = int(os.environ.get("KERNEL_NLAYERS", "3"))
            for l in range(_nl):
                de = IN_E if l == 0 else DH
                he_src = he0 if l == 0 else he_nxt[l]
                nc.sync.dma_start(attn_t[:], d_attn[l][:])

                def body(g, l=l, de=de, he_src=he_src):
                    # --- group loads ---
                    ilo = idxp.tile([P, WLO], I16, tag="ilo")
                    nc.sync.dma_start(ilo[:], d_idx_lo[:, ts(g, WLO)])
                    ihi = idxp.tile([P, WHI], I16, tag="ihi")
                    nc.sync.dma_start(ihi[:], d_idx_hi[:, ts(g, WHI)])
                    inj = idxp.tile([P, WNJ], I16, tag="inj")
                    nc.sync.dma_start(inj[:], d_idx_nj[:, ts(g, WNJ)])

                    glo = gb.tile([P, G * LO_TILES, 2 * DH], BF, tag="glo")
                    nc.gpsimd.dma_gather(
                        glo[:], full[l][0:(NCORES // 2) * s_core, :], ilo[:],
                        QLO, QLO, 2 * DH, single_packet=False,
                    )
                    ghi = gb.tile([P, G * HI_TILES, 2 * DH], BF, tag="ghi")
                    nc.gpsimd.dma_gather(
                        ghi[:], full[l][(NCORES // 2) * s_core:, :], ihi[:],
                        QHI, QHI, 2 * DH, single_packet=False,
                    )
                    gnj = gb.tile([P, G * TILES, 2 * DH], BF, tag="gnj")
                    nc.gpsimd.dma_gather(
                        gnj[:], fnj[l][:], inj[:], QNJ, QNJ, 2 * DH,
                        single_packet=False,
                    )
                    for cc in range(G):
                        heT = wk.tile([de, ECH], BF, tag="heT")
                        nc.sync.dma_start(
                            heT[:], he_src[:, ds((g * G + cc) * ECH, ECH)]
                        )
                        f_sb = wk.tile([P, TILES * DH], BF, tag="f_sb")
                        fT = wk.tile([DH, ECH], BF, tag="fT")
                        # ---------- loop A: per-tile f ----------
                        for t in range(TILES):
                            fps = pfs.tile([P, DH], F32, tag="pe")
                            nc.tensor.matmul(
                                fps[:],
                                lhsT=heT[:, t * P:(t + 1) * P],
                                rhs=wfij[l][:], start=True, stop=True,
                            )
                            if t < LO_TILES:
                                gt = glo[:, cc * LO_TILES + t, :]
                            else:
                                gt = ghi[:, cc * HI_TILES + (t - LO_TILES), :]
                            x1 = sm.tile([P, DH], F32, tag="x1")
                            nc.vector.tensor_add(out=x1[:], in0=fps[:],
                                                 in1=gt[:, 0:DH])
                            nc.vector.tensor_add(
                                out=x1[:], in0=x1[:],
                                in1=gnj[:, cc * TILES + t, 0:DH],
                            )
                            fsl = f_sb[:, t * DH:(t + 1) * DH]
                            # fsl = -leaky_relu(x) = -0.99*relu(-x) - x
                            rneg = sm.tile([P, DH], F32, tag="rneg")
                            nc.scalar.activation(rneg[:], x1[:], Act.Relu,
                                                 scale=-1.0)
                            nc.vector.scalar_tensor_tensor(
                                out=fsl, in0=rneg[:], scalar=-0.99, in1=x1[:],
                                op0=Alu.mult, op1=Alu.subtract,
                            )
                            relp = sm.tile([P, DH], BF, tag="relp")
                            nc.scalar.activation(relp[:], x1[:], Act.Relu)
                            tp = pfs.tile([DH, P], BF, tag="pe")
                            nc.tensor.transpose(tp[:], relp[:], ident[:])
                            nc.scalar.activation(
                                fT[:, t * P:(t + 1) * P], tp[:], Act.Copy,
                            )

                        # ---------- chunk-wide attention ----------
                        emul = sm.tile([P, TILES * DH], F32, tag="emul")
                        nc.vector.tensor_mul(
                            out=emul[:],
                            in0=f_sb[:],
                            in1=attn_t[:],
                        )
                        e_sb = sm.tile([P, TILES * NH], F32, tag="e_sb")
                        nc.vector.reduce_sum(
                            e_sb[:],
                            emul[:].rearrange("p (a b) -> p a b", b=HE),
                            axis=mybir.AxisListType.X,
                        )
                        nc.vector.tensor_scalar(
                            out=e_sb[:], in0=e_sb[:], scalar1=E_CLAMP,
                            scalar2=None, op0=Alu.min,
                        )
                        payload = sm.tile([P, TILES * (2 + DH)], BF, tag="payload")
                        pay3 = payload[:].rearrange(
                            "p (t f) -> p t f", f=2 + DH)
                        nc.scalar.activation(
                            pay3[:, :, 0:2],
                            e_sb[:].rearrange("p (t h) -> p t h", h=NH), Act.Exp
                        )
                        # wmsg per head (broadcast ex over 32 cols)
                        for t in range(TILES):
                            gt = (glo[:, cc * LO_TILES + t, :] if t < LO_TILES
                                  else ghi[:, cc * HI_TILES + (t - LO_TILES), :])
                            for h in range(NH):
                                nc.vector.tensor_mul(
                                    out=pay3[:, t, 2 + h * HE:2 + (h + 1) * HE],
                                    in0=gt[:, DH + h * HE:DH + (h + 1) * HE],
                                    in1=pay3[:, t, h:h + 1].to_broadcast([P, HE]),
                                )

                        # ---------- layer-2 MLP on fT ----------
                        if l == 2:
                            hidT = sm.tile([DH, ECH], BF, tag="hidT")
                            for off in range(0, ECH, 512):
                                w = min(512, ECH - off)
                                hp = paux.tile([DH, 512], F32, tag="paux")
                                nc.tensor.matmul(
                                    hp[:, :w], lhsT=w0[:],
                                    rhs=fT[:, off:off + w],
                                    start=True, stop=True,
                                )
                                t1 = sm.tile([DH, 512], F32, tag="t1")
                                nc.scalar.activation(
                                    t1[:, :w], hp[:, :w], Act.Exp, bias=b0[:]
                                )
                                t2 = sm.tile([DH, 512], F32, tag="t2")
                                nc.scalar.activation(
                                    t2[:, :w], hp[:, :w], Act.Relu, bias=b0[:]
                                )
                                r = sm.tile([DH, 512], F32, tag="r")
                                nc.scalar.activation(
                                    r[:, :w], t1[:, :w], Act.Relu,
                                    scale=-1.0, bias=1.0,
                                )
                                dtmp = sm.tile([DH, 512], F32, tag="dtmp")
                                nc.vector.scalar_tensor_tensor(
                                    out=dtmp[:, :w], in0=r[:, :w], scalar=SELU_A,
                                    in1=t2[:, :w], op0=Alu.mult, op1=Alu.subtract,
                                )
                                nc.vector.tensor_scalar(
                                    out=hidT[:, off:off + w], in0=dtmp[:, :w],
                                    scalar1=-SELU_L, scalar2=None, op0=Alu.mult,
                                )
                            inc_sb = sm.tile([P, TILES], F32, tag="inc_sb")
                            for t in range(TILES):
                                ip = paux.tile([P, 1], F32, tag="paux")
                                nc.tensor.matmul(
                                    ip[:], lhsT=hidT[:, t * P:(t + 1) * P],
                                    rhs=w1[:], start=True, stop=True,
                                )
                                nc.vector.tensor_scalar(
                                    out=inc_sb[:, t:t + 1], in0=ip[:],
                                    scalar1=b1[:, 0:1], scalar2=None,
                                    op0=Alu.add,
                                )
                            inc_bf = sm.tile([P, TILES], BF, tag="inc_bf")
                            nc.vector.tensor_copy(out=inc_bf[:], in_=inc_sb[:])

                        # ---------- loop B: scatter ----------
                        psc = pscat.tile([P, 2 + DH], F32, tag="psc")
                        if l == 2:
                            pD = pdel.tile([P, 1], F32, tag="pD")
                        for t in range(TILES):
                            oh = ohp.tile([P, P], BF, tag="oh")
                            col = g * (G * TILES) + cc * TILES + t
                            nc.vector.tensor_tensor(
                                out=oh[:],
                                in0=dstloc[:, ds(col, 1)].to_broadcast([P, P]),
                                in1=iota[:], op=Alu.is_equal,
                            )
                            nc.tensor.matmul(
                                psc[:], lhsT=oh[:],
                                rhs=payload[:, t * (2 + DH):(t + 1) * (2 + DH)],
                                start=(t == 0), stop=(t == TILES - 1),
                            )
                            if l == 2:
                                nc.tensor.matmul(
                                    pD[:], lhsT=oh[:], rhs=inc_bf[:, t:t + 1],
                                    start=(t == 0), stop=(t == TILES - 1),
                                )

                        # ---------- chunk tail ----------
                        if l < 2:
                            zt = sm.tile([P, 2], F32, tag="zt")
                            nc.vector.tensor_scalar(
                                out=zt[:], in0=psc[:, 0:2], scalar1=0.0,
                                scalar2=None, op0=Alu.is_equal,
                            )
                            s_sb = sm.tile([P, 2], F32, tag="s_sb")
                            nc.vector.tensor_add(out=s_sb[:], in0=psc[:, 0:2],
                                                 in1=zt[:])
                            rec = sm.tile([P, 2], F32, tag="rec")
                            nc.vector.reciprocal(rec[:], s_sb[:])
                            hr = sm.tile([P, DH], F32, tag="hr")
                            nc.scalar.activation(hr[:], psc[:, 2:2 + DH], Act.Relu)
                            hn_t = sm.tile([P, DH], BF, tag="hn_t")
                            for h in range(NH):
                                nc.vector.tensor_mul(
                                    out=hn_t[:, h * HN:(h + 1) * HN],
                                    in0=hr[:, h * HN:(h + 1) * HN],
                                    in1=rec[:, h:h + 1].to_broadcast([P, HN]),
                                )
                            project_and_stage(hn_t, g * G + cc, l + 1)
                        else:
                            oc = sm.tile([P, 1], F32, tag="oc")
                            nc.vector.tensor_add(
                                out=oc[:], in0=pD[:],
                                in1=chg_sb[:, ds(g * G + cc, 1)],
                            )
                            nc.vector.tensor_copy(
                                out=out_all[:, ds(g * G + cc, 1)], in_=oc[:]
                            )
                            nc.sync.dma_start(
                                out_inc[:, ds((g * G + cc) * TILES, TILES)],
                                inc_sb[:],
                            )

                        if l < 2:
                            nc.sync.dma_start(
                                he_nxt[l + 1][:, ds((g * G + cc) * ECH, ECH)],
                                fT[:],
                            )

                tc.For_i_unrolled(0, ngr, 1, body, max_unroll=4)

                if l < 2:
                    nc.gpsimd.collective_compute(
                        "AllGather", Alu.bypass, replica_groups=groups,
                        ins=[stage[l + 1][:]], outs=[full[l + 1][:]],
                    )

            nc.sync.dma_start(out_chg[:], out_all[:])

    return nc


# ======================================================================
# Entry point
# ======================================================================
_CACHE = {}


def kernel(feats_node, feats_edge, charges_init, src, dst, params):
    feats_node = np.asarray(feats_node, np.float32)
    feats_edge = np.asarray(feats_edge, np.float32)
    charges_init = np.asarray(charges_init, np.float32)
    src_i = np.asarray(src).astype(np.int64)
    dst_i = np.asarray(dst).astype(np.int64)
    params = [
        {k: np.asarray(v, np.float32) for k, v in p.items()} for p in params
    ]

    lay = _prep(src_i, dst_i)
    n_chunks, s_core = lay["n_chunks"], lay["s_core"]
    e_slots = n_chunks * ECH

    # ---- shared weight arrays ----
    wmaps = {}
    bf = ml_dtypes.bfloat16
    for l in range(3):
        p = params[l]
        wmaps[f"wfij{l}"] = np.ascontiguousarray(p["fc_fij"]).astype(bf)
        wmaps[f"w3_{l}"] = np.ascontiguousarray(
            np.concatenate([p["fc_ni"], p["fc_node"], p["fc_nj"]], axis=1)
        ).astype(bf)
        wmaps[f"bias{l}"] = np.tile(p["bias"][None, :], (P, 1)).astype(np.float32)
        attn_flat = -p["attn"].reshape(NH * HE)  # negated: f_sb holds -lrelu
        wmaps[f"attn{l}"] = np.tile(attn_flat[None, :], (P, TILES)).astype(
            np.float32
        )
    mlp = params[3]
    wmaps["w0"] = mlp["w0"].astype(bf)
    wmaps["b0"] = mlp["b0"].reshape(DH, 1)
    wmaps["w1"] = mlp["w1"].reshape(DH, 1).astype(bf)
    wmaps["b1"] = np.tile(mlp["b1"].reshape(1, 1), (P, 1))

    # ---- per-core arrays ----
    in_maps = []
    for c in range(NCORES):
        co = lay["cores"][c]
        eid = co["eid_of_slot"]
        val = eid >= 0
        he0 = np.zeros((e_slots, IN_E), np.float32)
        he0[val] = feats_edge[eid[val]]
        hn0 = np.zeros((s_core, DH), np.float32)
        nos = lay["node_of_slot"][c * s_core:(c + 1) * s_core]
        nval = nos >= 0
        hn0[nval] = feats_node[nos[nval]]
        chg_arr = np.zeros((s_core,), np.float32)
        chg_arr[nval] = charges_init[nos[nval]]
        m = dict(wmaps)
        m["he0"] = np.ascontiguousarray(he0.T).astype(ml_dtypes.bfloat16)
        m["hn0"] = hn0.astype(ml_dtypes.bfloat16)
        # chg layout [p, k] = slot k*P+p
        m["chg"] = np.ascontiguousarray(chg_arr.reshape(n_chunks, P).T)
        m["idx_lo"] = co["idx_lo"]
        m["idx_hi"] = co["idx_hi"]
        m["idx_nj"] = co["idx_nj"]
        m["dst_local"] = co["dst_local"]
        in_maps.append(m)

    # ---- build + compile (cache on n_chunks) ----
    if n_chunks not in _CACHE:
        nc = bacc.Bacc("TRN2", target_bir_lowering=False, debug=False,
                       num_devices=NCORES)
        build_program(nc, n_chunks)
        nc.compile()
        _CACHE[n_chunks] = nc
    nc = _CACHE[n_chunks]

    import os as _os
    _trace = bool(_os.environ.get("KERNEL_TRACE"))
    res = run_bass_kernel_spmd(
        nc, in_maps, core_ids=list(range(NCORES)), trace=_trace,
        tmpdir=_os.environ.get("KERNEL_TRACE_DIR") or None,
    )
    if _trace and res.exec_time_ns is not None:
        print(f"HW exec time: {res.exec_time_ns} ns")

    # ---- unshard ----
    out = np.zeros(N_NODES, np.float32)
    inc_full = np.zeros(N_EDGES, np.float32)
    for c in range(NCORES):
        r = res.results[c]
        co = lay["cores"][c]
        # out_chg [P, n_chunks] -> slots k*P+p
        oc = np.ascontiguousarray(r["out_chg"].T).reshape(-1)  # [s_core]
        nos = lay["node_of_slot"][c * s_core:(c + 1) * s_core]
        nval = nos >= 0
        out[nos[nval]] = oc[nval]
        # out_inc [P, n_chunks*TILES]: [p, k*TILES+t] = edge slot k*ECH+t*P+p
        oi = r["out_inc"].reshape(P, n_chunks, TILES).transpose(1, 2, 0).reshape(-1)
        eid = co["eid_of_slot"]
        ev = eid >= 0
        inc_full[eid[ev]] = oi[ev]

    out -= np.bincount(src_i, weights=inc_full, minlength=N_NODES).astype(
        np.float32
    )
    return out
